# revision 37
# baseline (speedup 1.0000x reference)
"""Multi-head attention (B=2, S=2048, H=16, D=64) on 8 TRN2 NeuronCores.

Sharding: data parallel on batch (2) x tensor parallel on heads (16 -> 4 per
core).  Core c handles batch c//4 and heads [4*(c%4), 4*(c%4)+4).  Each core
projects q/k/v for its head group from its batch's activations, runs the
full S x S attention for its 4 heads, and writes ctx in [head, D, S] layout.
The host transposes/concatenates shards (not part of HW exec time).

Device kernel (per core, identical SPMD program, no collectives).  The
scalar (ACT) engine is the limiting stream: softmax needs 16.8M
exps/core and ACT runs 1 elem/lane/cycle at 1.2 GHz (~136us busy across
128 x ~1.06us Exp ops); the PE carries ~150us of matmuls that must hide
underneath, so every scheduling trick below is about keeping both
pipes dense.

  - qT/kT in [D, S] layout, head pair packed into 128 partitions (head 2p
    on 0:64, head 2p+1 on 64:128).
  - scoresT per 128-key chunk via a row-tiled CONCURRENT matmul pair
    (tile_position (0,0)/(64,0), K=64 each): both heads' [128, 512]
    scores stream together in ~322ns/pair (vs 2x213 serial).
  - both outputs land in one [128, 2, 512] PSUM tile; ONE Exp (N=1024,
    scale=1/sqrt(D)) covers both heads.  PSUM is the hard wall on bigger
    Exps: sps(4 banks double-buffered) + ctx(2) + 2 proj tags = 8 banks.
  - softmax denominator via 64 ones-columns appended to v: the ctx matmul
    emits it free on psum partitions 64:128 (matmul cost is N-bound).
  - emission queues with due-patch indices: tasks lag 1 patch, ctx lags
    2.  The 2-patch ctx lag means its exp semaphore is satisfied before
    the PE reaches it, so the v weight-load pipelines instead of
    serializing after the wait (~160ns/patch saved).  Queues flush at
    window boundaries: normalize DVE reads must be EMITTED after the
    accumulation's last matmul (emission-order dep tracking; rolling
    them across windows races and corrupts intermittently).
  - pair-0 ramp interleaves qc0 (live ctx + JIT v projections) with qc1
    (exp only; e parked in SBUF, its ctx matmuls run as filler in qc2) to
    keep ACT fed while the PE grinds projections.
  - normalize chains are copy(psum den, partition-shifted) -> recip ->
    mult, queued and drained two DVE ops per patch.  Partition-shifted
    reads silently misread for every DVE op EXCEPT tensor_copy -- the
    shift must be done by a copy.
  - input DMA: host pre-shuffles x/w into chunk-major layouts so every
    descriptor is a 2-4KB contiguous partition line (512B lines cost 8x
    descriptors and ~30% aggregate bandwidth).  Dispatch order = need
    order, paced with tile_wait_until marks (absolute times; the
    framework preamble eats the first ~6.6us).  Tiny bias DMAs dispatch
    from the idle ACT sequencer.  f32 warm-up matmuls off a memset tile
    (no DMA dep) hold the PE clock at 2.4GHz until x0 lands.
  - padding mask folded into v_aug row zeroing (exp(x-1e4) underflows to
    0 in f32, so zeroing masked key rows is exactly equivalent).

History: 213.5us -> 188 -> ~183 (rel err 3.8e-3 vs 2e-2 budget).
Beware: the device throttles ~15% (to ~217us) under sustained back-to-
back runs; fresh-process runs after a compile pause measure true speed.
"""

import numpy as np
import ml_dtypes

import concourse.bass as bass
import concourse.tile as tile
from concourse import bacc, mybir
from concourse.bass_utils import run_bass_kernel_spmd

B, S, H, D = 2, 2048, 16, 64
HID = H * D
NCORES = 8
HPC = 4               # heads per core
COLS = HPC * D        # 256 projection columns per core
KC = HID // 128       # 8 contraction chunks for projections
QC = S // 512         # 4 query chunks of 512
MC = S // 128         # 16 key chunks of 128

BF16 = mybir.dt.bfloat16
F32 = mybir.dt.float32
np_bf16 = ml_dtypes.bfloat16

_CACHE = {}


def build(apply_mask: bool) -> bass.Bass:
    nc = bacc.Bacc(None, target_bir_lowering=False, debug=False)

    xT = nc.declare_dram_parameter("xT", [8, 128, KC, 256], BF16, isOutput=False)
    wq = nc.declare_dram_parameter("wq", [2, 128, KC, 128], BF16, isOutput=False)
    wk = nc.declare_dram_parameter("wk", [2, 128, KC, 128], BF16, isOutput=False)
    wv = nc.declare_dram_parameter("wv", [128, KC, COLS], BF16, isOutput=False)
    bq = nc.declare_dram_parameter("bq", [128, 2], F32, isOutput=False)
    bk = nc.declare_dram_parameter("bk", [128, 2], F32, isOutput=False)
    bv = nc.declare_dram_parameter("bv", [128, COLS], F32, isOutput=False)
    if apply_mask:
        mm_in = nc.declare_dram_parameter("maskm", [128, MC], F32, isOutput=False)
    out_ext = nc.declare_dram_parameter("out", [HPC, D, S], F32, isOutput=True)

    with tile.TileContext(nc) as tc:
        with (
            tc.tile_pool(name="singles", bufs=1) as singles,
            tc.tile_pool(name="work", bufs=6) as work,
            tc.tile_pool(name="psum", bufs=2, space="PSUM") as psum,
        ):
            # ---- input DMA, strict priority order, one dma_start per tensor
            # chunk (each dispatch costs ~600ns of serial sequencer time and
            # concurrent DMAs share bandwidth, so order = need order).
            # SBUF layouts are chunk-major so every DMA line is 2-4KB
            # contiguous per partition (512B lines cost ~8x in descriptor
            # count and cut aggregate DMA bandwidth ~30%). ----
            # tiny bias DMAs dispatch from the (idle-at-start) ACT
            # sequencer so the Sync sequencer's first dispatch is x0
            # (each dispatch costs ~600ns of serial sequencer time)
            bv_sb = singles.tile([128, COLS], F32)
            nc.scalar.dma_start(out=bv_sb, in_=bv[:, :])
            bq_sb = singles.tile([128, 2], F32)
            nc.scalar.dma_start(out=bq_sb, in_=bq[:, :])
            bk_sb = singles.tile([128, 2], F32)
            nc.scalar.dma_start(out=bk_sb, in_=bk[:, :])
            if apply_mask:
                mm_sb = singles.tile([128, MC], F32)
                nc.scalar.dma_start(out=mm_sb, in_=mm_in[:, :])

            # wq_sb/wk_sb: [128, pair, kc, 128]; x_sb: [128, cc, kc, 256]
            wq_sb = singles.tile([128, 2, KC, 128], BF16)
            wk_sb = singles.tile([128, 2, KC, 128], BF16)
            wv_sb = singles.tile([128, KC, COLS], BF16)
            x_sb = singles.tile([128, 8, KC, 256], BF16)

            def dma_w(w_sb, w_ext):
                nc.sync.dma_start(out=w_sb, in_=w_ext[:, :, :])

            def dma_w_pair(w_sb, w_ext, p):
                nc.sync.dma_start(out=w_sb[:, p], in_=w_ext[p])

            def dma_x(cc):
                nc.sync.dma_start(out=x_sb[:, cc], in_=xT[cc])

            # stagger the big DMAs: SDMA engines fair-share across all
            # in-flight copies, so without pacing the first-needed tensor
            # completes as late as the last.  Wait marks give x0/wk0/wq0 full
            # bandwidth, then release the rest in need order.
            # (the framework preamble runs until ~6.6us, so the earliest
            # dispatch lands there; marks below are absolute kernel time)
            dma_x(0)
            dma_w_pair(wk_sb, wk, 0)
            dma_w_pair(wq_sb, wq, 0)
            with tc.tile_wait_until(0.0095):
                dma_x(1)
            with tc.tile_wait_until(0.0115):
                dma_x(2)
            with tc.tile_wait_until(0.0125):
                dma_x(3)
            with tc.tile_wait_until(0.0135):
                dma_w(wv_sb, wv)
            with tc.tile_wait_until(0.0150):
                dma_x(4)
            with tc.tile_wait_until(0.0165):
                dma_x(5)
            with tc.tile_wait_until(0.0180):
                dma_x(6)
            with tc.tile_wait_until(0.0195):
                dma_x(7)
                dma_w_pair(wk_sb, wk, 1)
                dma_w_pair(wq_sb, wq, 1)

            # HAM warm-up off a memset tile (no DMA dependency, starts
            # ~0.3us in) so the PE clock is ramped to 2.4GHz when the first
            # projections run; output is never read.  Sized to end near
            # x0+wk0 arrival (~4us).
            warm_in = singles.tile([128, 128], F32)
            nc.vector.memset(warm_in, 0.0)
            warm_ps = psum.tile([128, 512], F32, tag="projkq", bufs=1, name="warm_ps")
            for i in range(10):
                nc.tensor.matmul(warm_ps[:, 0:128], lhsT=warm_in,
                                 rhs=warm_in,
                                 start=(i == 0), stop=(i == 9))

            # v_aug: [128, key_chunk, head, 128]; cols 64:128 are ones columns,
            # so the ctx matmul emits the softmax denominator replicated into
            # psum partitions 64:128 at no extra cost (matmul cost is N-bound)
            v_aug = singles.tile([128, MC, HPC, 128], BF16)
            nc.vector.memset(v_aug[:, :, :, 64:128], 1.0)

            kT = singles.tile([128, 2, S], BF16)
            qT = singles.tile([128, 2, S], BF16)

            # ---- projections as contiguous tasks (~1us each).  PSUM
            # accumulation groups MUST be emitted contiguously: splitting a
            # group across task slots with other matmuls in between corrupts
            # the accumulation (hardware-observed).  kT/qT chunks are
            # narrowed to 256 columns so a whole task still fits inside one
            # exp slot. ----
            _ptag = [0]

            def mk_kqT_task(dst, w_sb, b_sb, p, c256):
                csl = slice(c256 * 256, (c256 + 1) * 256)

                def go():
                    # alternate psum tags so a task's WAR wait lands two
                    # tasks back (long drained) instead of on the previous
                    # task's DVE epilogue
                    tag = ("projkq", "projv")[_ptag[0] % 2]
                    _ptag[0] += 1
                    ps = psum.tile([128, 256], F32, tag=tag, bufs=1,
                                   name=f"pt{nc.next_id()}")
                    for kc in range(KC):
                        nc.tensor.matmul(
                            ps, lhsT=w_sb[:, p, kc, :],
                            rhs=x_sb[:, c256, kc, :],
                            start=(kc == 0), stop=(kc == KC - 1))
                    nc.vector.tensor_tensor(
                        out=dst[:, p, csl], in0=ps,
                        in1=b_sb[:, p:p + 1].to_broadcast([128, 256]),
                        op=mybir.AluOpType.add)
                return go

            def mk_kqT_halves(dst, w_sb, b_sb, p, c256):
                """Split a kq projection task into two ~436ns PE chunks that
                fit the per-patch slack: half A accumulates kc 0..3 and
                parks (psum+bias) in SBUF, half B accumulates kc 4..7 and
                fuses park+psum with one scalar_tensor_tensor."""
                csl = slice(c256 * 256, (c256 + 1) * 256)
                box = {}

                def goA():
                    tag = ("projkq", "projv")[_ptag[0] % 2]
                    _ptag[0] += 1
                    ps = psum.tile([128, 256], F32, tag=tag, bufs=1,
                                   name=f"ph{nc.next_id()}")
                    for kc in range(4):
                        nc.tensor.matmul(
                            ps, lhsT=w_sb[:, p, kc, :],
                            rhs=x_sb[:, c256, kc, :],
                            start=(kc == 0), stop=(kc == 3))
                    tmp = work.tile([128, 256], F32, tag="ptmp",
                                    name=f"pm{nc.next_id()}")
                    nc.vector.tensor_tensor(
                        out=tmp, in0=ps,
                        in1=b_sb[:, p:p + 1].to_broadcast([128, 256]),
                        op=mybir.AluOpType.add)
                    box["tmp"] = tmp

                def goB():
                    tag = ("projkq", "projv")[_ptag[0] % 2]
                    _ptag[0] += 1
                    ps = psum.tile([128, 256], F32, tag=tag, bufs=1,
                                   name=f"ph{nc.next_id()}")
                    for kc in range(4, KC):
                        nc.tensor.matmul(
                            ps, lhsT=w_sb[:, p, kc, :],
                            rhs=x_sb[:, c256, kc, :],
                            start=(kc == 4), stop=(kc == KC - 1))
                    nc.vector.scalar_tensor_tensor(
                        out=dst[:, p, csl], in0=box["tmp"], scalar=1.0,
                        in1=ps, op0=mybir.AluOpType.mult,
                        op1=mybir.AluOpType.add)
                return goA, goB

            def mk_v_task(mc):
                def go():
                    tag = ("projv", "projkq")[_ptag[0] % 2]
                    _ptag[0] += 1
                    ps = psum.tile([128, COLS], F32, tag=tag, bufs=1,
                                   name=f"pv{nc.next_id()}")
                    h128 = (mc % 2) * 128
                    for kc in range(KC):
                        nc.tensor.matmul(
                            ps, lhsT=x_sb[:, mc // 2, kc, h128:h128 + 128],
                            rhs=wv_sb[:, kc, :],
                            start=(kc == 0), stop=(kc == KC - 1))
                    nc.vector.tensor_tensor(
                        out=v_aug[:, mc, :, 0:64],
                        in0=ps[:, :].rearrange("p (h d) -> p h d", h=HPC),
                        in1=bv_sb.rearrange("p (h d) -> p h d", h=HPC),
                        op=mybir.AluOpType.add)
                    if apply_mask:
                        nc.vector.tensor_tensor(
                            out=v_aug[:, mc, :, :],
                            in0=v_aug[:, mc, :, :],
                            in1=mm_sb[:, mc:mc + 1, None]
                                .to_broadcast([128, HPC, 128]),
                            op=mybir.AluOpType.mult)
                return go

            def mk_v_halves(mc):
                box = {}

                def goA():
                    tag = ("projv", "projkq")[_ptag[0] % 2]
                    _ptag[0] += 1
                    ps = psum.tile([128, COLS], F32, tag=tag, bufs=1,
                                   name=f"pva{nc.next_id()}")
                    h128 = (mc % 2) * 128
                    for kc in range(4):
                        nc.tensor.matmul(
                            ps, lhsT=x_sb[:, mc // 2, kc, h128:h128 + 128],
                            rhs=wv_sb[:, kc, :],
                            start=(kc == 0), stop=(kc == 3))
                    tmp = work.tile([128, COLS], F32, tag="ptmp",
                                    name=f"vm{nc.next_id()}")
                    nc.vector.tensor_tensor(
                        out=tmp, in0=ps,
                        in1=bv_sb, op=mybir.AluOpType.add)
                    box["tmp"] = tmp

                def goB():
                    tag = ("projv", "projkq")[_ptag[0] % 2]
                    _ptag[0] += 1
                    ps = psum.tile([128, COLS], F32, tag=tag, bufs=1,
                                   name=f"pvb{nc.next_id()}")
                    h128 = (mc % 2) * 128
                    for kc in range(4, KC):
                        nc.tensor.matmul(
                            ps, lhsT=x_sb[:, mc // 2, kc, h128:h128 + 128],
                            rhs=wv_sb[:, kc, :],
                            start=(kc == 4), stop=(kc == KC - 1))
                    nc.vector.scalar_tensor_tensor(
                        out=v_aug[:, mc, :, 0:64],
                        in0=box["tmp"].rearrange("p (h d) -> p h d", h=HPC),
                        scalar=1.0,
                        in1=ps[:, :].rearrange("p (h d) -> p h d", h=HPC),
                        op0=mybir.AluOpType.mult, op1=mybir.AluOpType.add)
                    if apply_mask:
                        nc.vector.tensor_tensor(
                            out=v_aug[:, mc, :, :],
                            in0=v_aug[:, mc, :, :],
                            in1=mm_sb[:, mc:mc + 1, None]
                                .to_broadcast([128, HPC, 128]),
                            op=mybir.AluOpType.mult)
                return goA, goB

            # deferred-e store for the ramp (pair-0 qc1 exps run during qc0's
            # projection-heavy window; their ctx matmuls run later in qc2)
            e_defer = singles.tile([128, MC, 2, 512], BF16)
            e_defer0 = singles.tile([128, MC, 2, 512], BF16)

            def fill_mms(p, qc, kc, s):
                qsl = slice(qc * 512, (qc + 1) * 512)
                ksl = slice(kc * 128, (kc + 1) * 128)
                nc.tensor.matmul(s[:, 0, :], lhsT=kT[0:64, p, ksl],
                                 rhs=qT[0:64, p, qsl], start=True, stop=True)
                nc.tensor.matmul(s[:, 1, :], lhsT=kT[64:128, p, ksl],
                                 rhs=qT[64:128, p, qsl], start=True, stop=True)

            def ctx_mms(p, kc, e_ap, ctx_a, ctx_b, start, stop):
                ha, hb = 2 * p, 2 * p + 1
                nc.tensor.matmul(ctx_a, lhsT=v_aug[:, kc, ha, :],
                                 rhs=e_ap[0], start=start, stop=stop)
                nc.tensor.matmul(ctx_b, lhsT=v_aug[:, kc, hb, :],
                                 rhs=e_ap[1], start=start, stop=stop)

            def mk_ctx(p, qc):
                return (psum.tile([128, 512], F32, tag="ctx",
                                  name=f"ca{p}{qc}{nc.next_id()}"),
                        psum.tile([128, 512], F32, tag="ctx",
                                  name=f"cb{p}{qc}{nc.next_id()}"))

            import collections
            norm_q = collections.deque()

            def normalize_steps(p, qc, ctx_pair):
                """6 DVE closures (2 chains of 3) queued for spreading,
                drained two per subsequent patch, so the DVE never bursts
                and proj-slot WAR waits stay short.  Partition-shifted
                reads (den on psum rows 64:128 -> lanes 0:64) only work
                SBUF->SBUF, so the chain copies out of PSUM unshifted
                first."""
                ha = 2 * p
                qsl = slice(qc * 512, (qc + 1) * 512)
                for h, ctx in ((ha, ctx_pair[0]), (ha + 1, ctx_pair[1])):
                    box = {}

                    def s1(ctx=ctx, box=box):
                        d0 = work.tile([64, 512], F32, tag="den0",
                                       name=f"d0{nc.next_id()}")
                        nc.vector.tensor_copy(out=d0, in_=ctx[64:128, :])
                        box["d0"] = d0

                    def s2(box=box):
                        d = work.tile([64, 512], F32, tag="den",
                                      name=f"d{nc.next_id()}")
                        nc.vector.reciprocal_approx_fast(
                            out=d, in_=box["d0"])
                        box["d"] = d

                    def s3(h=h, ctx=ctx, box=box):
                        o = work.tile([64, 512], F32, tag="outt",
                                      name=f"o{nc.next_id()}")
                        nc.vector.tensor_tensor(out=o, in0=ctx[0:64, :],
                                                in1=box["d"],
                                                op=mybir.AluOpType.mult)
                        nc.sync.dma_start(out=out_ext[h][:, qsl], in_=o)

                    for s in (s1, s2, s3):
                        norm_q.append(s)

            def drain_norm(k=None):
                n = len(norm_q) if k is None else min(k, len(norm_q))
                for _ in range(n):
                    norm_q.popleft()()

            # Software-pipelined emission: PE stream per patch i is
            #   fill(i), tasks(i-1), ctx(i-2), fill(i+1), tasks(i), ctx(i-1)
            # Tasks have no exp dependency (they pre-run during exps).
            # ctx lags TWO patches so its exp semaphore is long satisfied
            # when the PE reaches it: the weight load pipelines instead of
            # serializing after the wait (~160ns/patch) and the PE never
            # idles on the exp.  e tiles are 4-buffered so a 2-patch-old e
            # is still live.
            # Deferred-emission queues: entries are (due_patch, closure).
            # tasks run one patch after their emission point (lag 1), ctx
            # pairs two (lag 2).  The queues roll ACROSS window boundaries
            # (no flush) so the PE never gets a bunched backlog that would
            # stall the next window's first fill.  A window's normalize is
            # queued INTO ctxq right behind its stop matmul: the Tile
            # framework tracks deps at emission time, so the norm reads
            # must be emitted after the accumulation's last write.
            taskq = collections.deque()
            ctxq = collections.deque()
            gp = [0]

            def flush_all():
                while taskq:
                    taskq.popleft()[1]()
                while ctxq:
                    ctxq.popleft()[1]()

            def flush_pending():
                flush_all()
                drain_norm(2)

            def patch(p, qc, kc, ctx_pair, tasks, e_dst=None):
                """ctx_pair=None -> exp only (e parked in e_dst)."""
                if e_dst is None:
                    e_dst = work.tile([128, 2, 512], BF16, tag="expT",
                                      name=f"e{nc.next_id()}")
                s = psum.tile([128, 2, 512], F32, tag="sps",
                              name=f"s{nc.next_id()}")
                fill_mms(p, qc, kc, s)
                last = kc == MC - 1 and ctx_pair is not None
                if last:
                    # final patch of a live window: emit the exp FIRST so
                    # this very patch's ctx can be emitted in-window (it
                    # may only be emitted after its exp, and it executes
                    # in the exp's shadow) -- keeps the boundary flush to
                    # just tasks(15)
                    nc.scalar.activation(e_dst, s,
                                         mybir.ActivationFunctionType.Exp,
                                         scale=0.125)
                    ctxq.append(
                        (gp[0],
                         lambda p=p, kc=kc, e_dst=e_dst, ctx_pair=ctx_pair:
                         ctx_mms(p, kc,
                                 (e_dst[:, 0, :], e_dst[:, 1, :]),
                                 *ctx_pair, start=False, stop=True)))
                while taskq and taskq[0][0] <= gp[0]:
                    taskq.popleft()[1]()
                while ctxq and ctxq[0][0] <= gp[0] + (kc == MC - 1):
                    ctxq.popleft()[1]()
                drain_norm(2)
                if not last:
                    nc.scalar.activation(e_dst, s,
                                         mybir.ActivationFunctionType.Exp,
                                         scale=0.125)
                taskq.append(
                    (gp[0] + 1, lambda tasks=tasks: [t() for t in tasks]))
                if ctx_pair is not None and not last:
                    ctxq.append(
                        (gp[0] + 2,
                         lambda p=p, kc=kc, e_dst=e_dst, ctx_pair=ctx_pair:
                         ctx_mms(p, kc,
                                 (e_dst[:, 0, :], e_dst[:, 1, :]),
                                 *ctx_pair, start=(kc == 0),
                                 stop=(kc == MC - 1))))
                gp[0] += 1

            # ---- task schedule ----
            # kq[(t, p, c256)]: 256-column kT/qT projection task
            kq = {}
            for t, dst, w_sb, b_sb in (("kT", kT, wk_sb, bk_sb),
                                       ("qT", qT, wq_sb, bq_sb)):
                for p in range(2):
                    for c in range(8):
                        kq[(t, p, c)] = mk_kqT_task(dst, w_sb, b_sb, p, c)
            v_t = {mc: mk_v_task(mc) for mc in range(MC)}

            # pair-0 minimal prefix: keys 0:256 and the qc0 queries (v is
            # first consumed in qc2 now that qc0's ctx is deferred, so v0
            # rides the ramp instead of the serial prefix)
            kq[("kT", 0, 0)]()
            kq[("qT", 0, 0)]()
            kq[("qT", 0, 1)]()

            # ---- pair-0 ramp: qc0 patches 0-4 solo, then interleave with
            # qc1 (exp only, e parked in SBUF; its ctx runs in qc2), then
            # qc1 tail.  One task per patch; each task is placed at the
            # earliest patch whose input DMA (x chunk / wv) has surely
            # landed, x-gated kT/qT chunks first (their fill deadlines are
            # hard), v chunks in the remaining slots (first consumed in
            # qc2).
            ramp = [
                (0, 0, [kq[("kT", 0, 1)]]),
                (0, 1, [kq[("kT", 0, 2)]]),
                (0, 2, [kq[("qT", 0, 2)]]),
                (0, 3, [kq[("qT", 0, 3)]]),
                (0, 4, [kq[("kT", 0, 3)]]),
            ]
            qc1_tasks = [
                [v_t[5]], [kq[("kT", 0, 4)]], [kq[("kT", 0, 5)]],
                [kq[("kT", 0, 6)]], [kq[("kT", 0, 7)]],
                [kq[("qT", 0, 4)]], [kq[("qT", 0, 5)]],
                [kq[("qT", 0, 6)]], [kq[("qT", 0, 7)]],
                [v_t[12]], [v_t[13]],
            ]
            qc0_tail = [
                [v_t[6]], [v_t[7]], [v_t[8]], [v_t[9]], [v_t[10]],
                [v_t[11]], [v_t[14]], [v_t[15]], [v_t[0]], [v_t[1]],
                [v_t[2]],
            ]
            v3A, v3B = mk_v_halves(3)
            v4A, v4B = mk_v_halves(4)
            for i in range(11):
                ramp.append((1, i, qc1_tasks[i]))
                ramp.append((0, 5 + i, qc0_tail[i]))
            for i, t in ((11, [v3A]), (12, [v3B]), (13, [v4A]),
                         (14, [v4B]), (15, [])):
                ramp.append((1, i, t))

            for qc, kc, tasks in ramp:
                if qc == 0:
                    patch(0, 0, kc, None, tasks,
                          e_dst=e_defer0[:, kc, :, :])
                else:
                    patch(0, 1, kc, None, tasks, e_dst=e_defer[:, kc, :, :])
            flush_pending()

            # qc2: own patches + qc1's deferred ctx matmuls (2 per patch)
            ctx1 = (psum.tile([128, 512], F32, tag="projkq", bufs=1,
                              name="dca"),
                    psum.tile([128, 512], F32, tag="projv", bufs=1,
                              name="dcb"))
            ctx2 = mk_ctx(0, 2)
            for kc in range(MC):
                patch(0, 2, kc, ctx2,
                      [lambda kc=kc: ctx_mms(
                          0, kc, (e_defer[:, kc, 0, :], e_defer[:, kc, 1, :]),
                          *ctx1, start=(kc == 0), stop=(kc == MC - 1))])
            flush_pending()
            normalize_steps(0, 2, ctx2)
            normalize_steps(0, 1, ctx1)

            # qc3: own patches + pair-1 kT + qT(1,0)
            # only the tasks pair-1 qc0 needs up-front ride here, on
            # alternating patches (consecutive task-patches overflow the
            # per-patch PE slack; alternating ones are absorbed by the
            # s-tile double buffer), late enough that the qc1/qc2
            # normalize drain (DVE) finishes first
            kqh = {}
            for t, dst, w_sb, b_sb in (("kT", kT, wk_sb, bk_sb),
                                       ("qT", qT, wq_sb, bq_sb)):
                for c in range(8):
                    kqh[(t, c)] = mk_kqT_halves(dst, w_sb, b_sb, 1, c)
            qc3_tasks = {4: kqh[("kT", 0)][0], 5: kqh[("kT", 0)][1],
                         6: kqh[("kT", 1)][0], 7: kqh[("kT", 1)][1],
                         8: kqh[("kT", 2)][0], 9: kqh[("kT", 2)][1],
                         10: kqh[("kT", 3)][0], 11: kqh[("kT", 3)][1],
                         12: kqh[("qT", 0)][0], 13: kqh[("qT", 0)][1],
                         14: kqh[("qT", 1)][0], 15: kqh[("qT", 1)][1]}
            ctx3 = mk_ctx(0, 3)
            for kc in range(MC):
                t = [qc3_tasks[kc]] if kc in qc3_tasks else []
                patch(0, 3, kc, ctx3, t)
            flush_pending()
            normalize_steps(0, 3, ctx3)

            # ---- pair 1 (kT tail + qT chunks ride alternating patches of
            # earlier windows; kT(1,4..7) are first needed at qc0 patch 8) ----
            p1_tasks = {
                (0, 1): [kq[("kT", 1, 4)]], (0, 3): [kq[("kT", 1, 5)]],
                (0, 5): [kq[("kT", 1, 6)]], (0, 7): [kq[("kT", 1, 7)]],
                (0, 9): [kqh[("qT", 2)][0]], (0, 11): [kqh[("qT", 2)][1]],
                (0, 13): [kq[("qT", 1, 3)]],
                (1, 1): [kqh[("qT", 4)][0]], (1, 3): [kqh[("qT", 4)][1]],
                (1, 5): [kqh[("qT", 5)][0]], (1, 7): [kqh[("qT", 5)][1]],
                (1, 9): [kqh[("qT", 6)][0]], (1, 11): [kqh[("qT", 6)][1]],
                (1, 13): [kqh[("qT", 7)][0]], (1, 15): [kqh[("qT", 7)][1]],
            }
            ctx00 = None
            for qc in range(QC):
                ctxp = mk_ctx(1, qc)
                if qc == 2:
                    # qc0's deferred ctx matmuls fill this window's PE slack
                    # (its proj psum tags are idle here)
                    ctx00 = (psum.tile([128, 512], F32, tag="projkq", bufs=1,
                                       name="d0a"),
                             psum.tile([128, 512], F32, tag="projv", bufs=1,
                                       name="d0b"))
                for kc in range(MC):
                    t = list(p1_tasks.get((qc, kc), []))
                    if qc == 2:
                        t.append(lambda kc=kc: ctx_mms(
                            0, kc,
                            (e_defer0[:, kc, 0, :], e_defer0[:, kc, 1, :]),
                            *ctx00, start=(kc == 0), stop=(kc == MC - 1)))
                    patch(1, qc, kc, ctxp, t)
                flush_pending()
                normalize_steps(1, qc, ctxp)
                if qc == 2:
                    normalize_steps(0, 0, ctx00)
            flush_all()
            drain_norm()

    nc.compile()
    return nc


def _get_nc(apply_mask: bool) -> bass.Bass:
    if apply_mask not in _CACHE:
        _CACHE[apply_mask] = build(apply_mask)
    return _CACHE[apply_mask]


def _shuf_w(w):
    # [HID, COLS] -> [2, 128, KC, 128]: pair-major, partition-contiguous
    return np.ascontiguousarray(
        w.reshape(KC, 128, 2, 128).transpose(2, 1, 0, 3)).astype(np_bf16)


def _shuf_w_flat(w):
    # [HID, COLS] -> [128, KC, COLS]: partition-contiguous DMA lines
    return np.ascontiguousarray(
        w.reshape(KC, 128, COLS).transpose(1, 0, 2)).astype(np_bf16)


def _in_maps(x, mask, Wq, bq, Wk, bk, Wv, bv, apply_mask):
    # x[b].T is [HID, S]; shuffle to [8, 128, KC, 256] (256-position
    # chunks, 1:1 with projection tasks) so each DMA reads 4KB contiguous
    # per partition and the first projections start as early as possible
    xT_b = [np.ascontiguousarray(
        x[b].T.reshape(KC, 128, 8, 256).transpose(2, 1, 0, 3)).astype(np_bf16)
        for b in range(B)]
    maps = []
    for c in range(NCORES):
        b, hg = c // 4, c % 4
        cs = slice(hg * COLS, (hg + 1) * COLS)
        m = {
            "xT": xT_b[b],
            "wq": _shuf_w(Wq[:, cs]),
            "wk": _shuf_w(Wk[:, cs]),
            "wv": _shuf_w_flat(Wv[:, cs]),
            "bq": np.ascontiguousarray(bq[cs].reshape(2, 128).T).astype(np.float32),
            "bk": np.ascontiguousarray(bk[cs].reshape(2, 128).T).astype(np.float32),
            "bv": np.ascontiguousarray(
                np.broadcast_to(bv[cs], (128, COLS))).astype(np.float32),
        }
        if apply_mask:
            m["maskm"] = np.ascontiguousarray(
                mask[b].astype(np.float32).reshape(MC, 128).T)
        maps.append(m)
    return maps


def _ensure_ntff_hook():
    """The agent image's antenv lacks axon_hooks; synthesize it so
    run_bass_kernel_spmd(trace=True) can reach the axon NTFF profiler."""
    import sys as _sys
    import types as _types
    try:
        from antenv import axon_hooks  # noqa: F401
        return
    except ImportError:
        pass
    import antenv
    mod = _types.ModuleType("antenv.axon_hooks")
    _hook = [None]
    mod.set_axon_ntff_profile_hook = lambda h: _hook.__setitem__(0, h)
    mod.get_axon_ntff_profile_hook = lambda: _hook[0]
    _sys.modules["antenv.axon_hooks"] = mod
    antenv.axon_hooks = mod
    from trn_agent_boot.trn_boot import _ntff_profile_via_ctypes
    mod.set_axon_ntff_profile_hook(
        _ntff_profile_via_ctypes("/opt/axon/libaxon_pjrt.so"))


def run(inputs: dict, trace: bool = False):
    if trace:
        _ensure_ntff_hook()
    x = np.asarray(inputs["x"], dtype=np.float32)
    mask = np.asarray(inputs["mask"])
    apply_mask = not bool((mask == 1).all())
    nc = _get_nc(apply_mask)
    maps = _in_maps(x, mask, np.asarray(inputs["Wq"], np.float32),
                    np.asarray(inputs["bq"], np.float32),
                    np.asarray(inputs["Wk"], np.float32),
                    np.asarray(inputs["bk"], np.float32),
                    np.asarray(inputs["Wv"], np.float32),
                    np.asarray(inputs["bv"], np.float32), apply_mask)
    res = run_bass_kernel_spmd(nc, maps, core_ids=list(range(NCORES)), trace=trace)
    out = np.empty((B, S, HID), dtype=np.float32)
    for c in range(NCORES):
        b, hg = c // 4, c % 4
        cs = slice(hg * COLS, (hg + 1) * COLS)
        ctxT = res.results[c]["out"]          # [HPC, D, S]
        out[b, :, cs] = ctxT.transpose(2, 0, 1).reshape(S, COLS)
    return out, res


def kernel(**inputs) -> np.ndarray:
    out, _ = run(inputs)
    return out



# revision 38
# speedup vs baseline: 1.0292x; 1.0292x over previous
"""Multi-head attention (B=2, S=2048, H=16, D=64) on 8 TRN2 NeuronCores.

Sharding: data parallel on batch (2) x tensor parallel on heads (16 -> 4 per
core).  Core c handles batch c//4 and heads [4*(c%4), 4*(c%4)+4).  Each core
projects q/k/v for its head group from its batch's activations, runs the
full S x S attention for its 4 heads, and writes ctx in [head, D, S] layout.
The host transposes/concatenates shards (not part of HW exec time).

Device kernel (per core, identical SPMD program, no collectives).  The
scalar (ACT) engine is the limiting stream: softmax needs 16.8M
exps/core and ACT runs 1 elem/lane/cycle at 1.2 GHz (~136us busy across
128 x ~1.06us Exp ops); the PE carries ~150us of matmuls that must hide
underneath, so every scheduling trick below is about keeping both
pipes dense.

  - qT/kT in [D, S] layout, head pair packed into 128 partitions (head 2p
    on 0:64, head 2p+1 on 64:128).
  - scoresT per 128-key chunk via a row-tiled CONCURRENT matmul pair
    (tile_position (0,0)/(64,0), K=64 each): both heads' [128, 512]
    scores stream together in ~322ns/pair (vs 2x213 serial).
  - both outputs land in one [128, 2, 512] PSUM tile; ONE Exp (N=1024,
    scale=1/sqrt(D)) covers both heads.  PSUM is the hard wall on bigger
    Exps: sps(4 banks double-buffered) + ctx(2) + 2 proj tags = 8 banks.
  - softmax denominator via 64 ones-columns appended to v: the ctx matmul
    emits it free on psum partitions 64:128 (matmul cost is N-bound).
  - emission queues with due-patch indices: tasks lag 1 patch, ctx lags
    2.  The 2-patch ctx lag means its exp semaphore is satisfied before
    the PE reaches it, so the v weight-load pipelines instead of
    serializing after the wait (~160ns/patch saved).  Queues flush at
    window boundaries: normalize DVE reads must be EMITTED after the
    accumulation's last matmul (emission-order dep tracking; rolling
    them across windows races and corrupts intermittently).
  - pair-0 ramp interleaves qc0 (live ctx + JIT v projections) with qc1
    (exp only; e parked in SBUF, its ctx matmuls run as filler in qc2) to
    keep ACT fed while the PE grinds projections.
  - normalize chains are copy(psum den, partition-shifted) -> recip ->
    mult, queued and drained two DVE ops per patch.  Partition-shifted
    reads silently misread for every DVE op EXCEPT tensor_copy -- the
    shift must be done by a copy.
  - input DMA: host pre-shuffles x/w into chunk-major layouts so every
    descriptor is a 2-4KB contiguous partition line (512B lines cost 8x
    descriptors and ~30% aggregate bandwidth).  Dispatch order = need
    order, paced with tile_wait_until marks (absolute times; the
    framework preamble eats the first ~6.6us).  Tiny bias DMAs dispatch
    from the idle ACT sequencer.  f32 warm-up matmuls off a memset tile
    (no DMA dep) hold the PE clock at 2.4GHz until x0 lands.
  - padding mask folded into v_aug row zeroing (exp(x-1e4) underflows to
    0 in f32, so zeroing masked key rows is exactly equivalent).

History: 213.5us -> 188 -> ~183 (rel err 3.8e-3 vs 2e-2 budget).
Beware: the device throttles ~15% (to ~217us) under sustained back-to-
back runs; fresh-process runs after a compile pause measure true speed.
"""

import numpy as np
import ml_dtypes

import concourse.bass as bass
import concourse.tile as tile
from concourse import bacc, mybir
from concourse.bass_utils import run_bass_kernel_spmd

B, S, H, D = 2, 2048, 16, 64
HID = H * D
NCORES = 8
HPC = 4               # heads per core
COLS = HPC * D        # 256 projection columns per core
KC = HID // 128       # 8 contraction chunks for projections
QC = S // 512         # 4 query chunks of 512
MC = S // 128         # 16 key chunks of 128

BF16 = mybir.dt.bfloat16
F32 = mybir.dt.float32
np_bf16 = ml_dtypes.bfloat16

_CACHE = {}


def build(apply_mask: bool) -> bass.Bass:
    nc = bacc.Bacc(None, target_bir_lowering=False, debug=False)

    xT = nc.declare_dram_parameter("xT", [8, 128, KC, 256], BF16, isOutput=False)
    wq = nc.declare_dram_parameter("wq", [2, 128, KC, 128], BF16, isOutput=False)
    wk = nc.declare_dram_parameter("wk", [2, 128, KC, 128], BF16, isOutput=False)
    wv = nc.declare_dram_parameter("wv", [128, KC, COLS], BF16, isOutput=False)
    bq = nc.declare_dram_parameter("bq", [128, 2], F32, isOutput=False)
    bk = nc.declare_dram_parameter("bk", [128, 2], F32, isOutput=False)
    bv = nc.declare_dram_parameter("bv", [128, COLS], F32, isOutput=False)
    if apply_mask:
        mm_in = nc.declare_dram_parameter("maskm", [128, MC], F32, isOutput=False)
    out_ext = nc.declare_dram_parameter("out", [HPC, D, S], F32, isOutput=True)

    with tile.TileContext(nc) as tc:
        with (
            tc.tile_pool(name="singles", bufs=1) as singles,
            tc.tile_pool(name="work", bufs=6) as work,
            tc.tile_pool(name="psum", bufs=2, space="PSUM") as psum,
        ):
            # ---- input DMA, strict priority order, one dma_start per tensor
            # chunk (each dispatch costs ~600ns of serial sequencer time and
            # concurrent DMAs share bandwidth, so order = need order).
            # SBUF layouts are chunk-major so every DMA line is 2-4KB
            # contiguous per partition (512B lines cost ~8x in descriptor
            # count and cut aggregate DMA bandwidth ~30%). ----
            # tiny bias DMAs dispatch from the (idle-at-start) ACT
            # sequencer so the Sync sequencer's first dispatch is x0
            # (each dispatch costs ~600ns of serial sequencer time)
            bv_sb = singles.tile([128, COLS], F32)
            nc.scalar.dma_start(out=bv_sb, in_=bv[:, :])
            bq_sb = singles.tile([128, 2], F32)
            nc.scalar.dma_start(out=bq_sb, in_=bq[:, :])
            bk_sb = singles.tile([128, 2], F32)
            nc.scalar.dma_start(out=bk_sb, in_=bk[:, :])
            if apply_mask:
                mm_sb = singles.tile([128, MC], F32)
                nc.scalar.dma_start(out=mm_sb, in_=mm_in[:, :])

            # wq_sb/wk_sb: [128, pair, kc, 128]; x_sb: [128, cc, kc, 256]
            wq_sb = singles.tile([128, 2, KC, 128], BF16)
            wk_sb = singles.tile([128, 2, KC, 128], BF16)
            wv_sb = singles.tile([128, KC, COLS], BF16)
            x_sb = singles.tile([128, 8, KC, 256], BF16)

            def dma_w(w_sb, w_ext):
                nc.sync.dma_start(out=w_sb, in_=w_ext[:, :, :])

            def dma_w_pair(w_sb, w_ext, p):
                nc.sync.dma_start(out=w_sb[:, p], in_=w_ext[p])

            def dma_x(cc):
                nc.sync.dma_start(out=x_sb[:, cc], in_=xT[cc])

            # stagger the big DMAs: SDMA engines fair-share across all
            # in-flight copies, so without pacing the first-needed tensor
            # completes as late as the last.  Wait marks give x0/wk0/wq0 full
            # bandwidth, then release the rest in need order.
            # (the framework preamble runs until ~6.6us, so the earliest
            # dispatch lands there; marks below are absolute kernel time)
            dma_x(0)
            dma_w_pair(wk_sb, wk, 0)
            dma_w_pair(wq_sb, wq, 0)
            with tc.tile_wait_until(0.0095):
                dma_x(1)
            with tc.tile_wait_until(0.0115):
                dma_x(2)
            with tc.tile_wait_until(0.0125):
                dma_x(3)
            with tc.tile_wait_until(0.0135):
                dma_w(wv_sb, wv)
            with tc.tile_wait_until(0.0150):
                dma_x(4)
            with tc.tile_wait_until(0.0165):
                dma_x(5)
            with tc.tile_wait_until(0.0180):
                dma_x(6)
            with tc.tile_wait_until(0.0195):
                dma_x(7)
                dma_w_pair(wk_sb, wk, 1)
                dma_w_pair(wq_sb, wq, 1)

            # HAM warm-up off a memset tile (no DMA dependency, starts
            # ~0.3us in) so the PE clock is ramped to 2.4GHz when the first
            # projections run; output is never read.  Sized to end near
            # x0+wk0 arrival (~4us).
            warm_in = singles.tile([128, 128], F32)
            nc.vector.memset(warm_in, 0.0)
            warm_ps = psum.tile([128, 512], F32, tag="projkq", bufs=1, name="warm_ps")
            for i in range(10):
                nc.tensor.matmul(warm_ps[:, 0:128], lhsT=warm_in,
                                 rhs=warm_in,
                                 start=(i == 0), stop=(i == 9))

            # v_aug: [128, key_chunk, head, 128]; cols 64:128 are ones columns,
            # so the ctx matmul emits the softmax denominator replicated into
            # psum partitions 64:128 at no extra cost (matmul cost is N-bound)
            v_aug = singles.tile([128, MC, HPC, 128], BF16)
            nc.vector.memset(v_aug[:, :, :, 64:128], 1.0)

            kT = singles.tile([128, 2, S], BF16)
            qT = singles.tile([128, 2, S], BF16)

            # ---- projections as contiguous tasks (~1us each).  PSUM
            # accumulation groups MUST be emitted contiguously: splitting a
            # group across task slots with other matmuls in between corrupts
            # the accumulation (hardware-observed).  kT/qT chunks are
            # narrowed to 256 columns so a whole task still fits inside one
            # exp slot. ----
            _ptag = [0]

            def mk_kqT_task(dst, w_sb, b_sb, p, c256):
                csl = slice(c256 * 256, (c256 + 1) * 256)

                def go():
                    # alternate psum tags so a task's WAR wait lands two
                    # tasks back (long drained) instead of on the previous
                    # task's DVE epilogue
                    tag = ("projkq", "projv")[_ptag[0] % 2]
                    _ptag[0] += 1
                    ps = psum.tile([128, 256], F32, tag=tag, bufs=1,
                                   name=f"pt{nc.next_id()}")
                    for kc in range(KC):
                        nc.tensor.matmul(
                            ps, lhsT=w_sb[:, p, kc, :],
                            rhs=x_sb[:, c256, kc, :],
                            start=(kc == 0), stop=(kc == KC - 1))
                    nc.vector.tensor_tensor(
                        out=dst[:, p, csl], in0=ps,
                        in1=b_sb[:, p:p + 1].to_broadcast([128, 256]),
                        op=mybir.AluOpType.add)
                return go

            def mk_kqT_halves(dst, w_sb, b_sb, p, c256):
                """Split a kq projection task into two ~436ns PE chunks that
                fit the per-patch slack: half A accumulates kc 0..3 and
                parks (psum+bias) in SBUF, half B accumulates kc 4..7 and
                fuses park+psum with one scalar_tensor_tensor."""
                csl = slice(c256 * 256, (c256 + 1) * 256)
                box = {}

                def goA():
                    tag = ("projkq", "projv")[_ptag[0] % 2]
                    _ptag[0] += 1
                    ps = psum.tile([128, 256], F32, tag=tag, bufs=1,
                                   name=f"ph{nc.next_id()}")
                    for kc in range(4):
                        nc.tensor.matmul(
                            ps, lhsT=w_sb[:, p, kc, :],
                            rhs=x_sb[:, c256, kc, :],
                            start=(kc == 0), stop=(kc == 3))
                    tmp = work.tile([128, 256], F32, tag="ptmp",
                                    name=f"pm{nc.next_id()}")
                    nc.vector.tensor_tensor(
                        out=tmp, in0=ps,
                        in1=b_sb[:, p:p + 1].to_broadcast([128, 256]),
                        op=mybir.AluOpType.add)
                    box["tmp"] = tmp

                def goB():
                    tag = ("projkq", "projv")[_ptag[0] % 2]
                    _ptag[0] += 1
                    ps = psum.tile([128, 256], F32, tag=tag, bufs=1,
                                   name=f"ph{nc.next_id()}")
                    for kc in range(4, KC):
                        nc.tensor.matmul(
                            ps, lhsT=w_sb[:, p, kc, :],
                            rhs=x_sb[:, c256, kc, :],
                            start=(kc == 4), stop=(kc == KC - 1))
                    nc.vector.scalar_tensor_tensor(
                        out=dst[:, p, csl], in0=box["tmp"], scalar=1.0,
                        in1=ps, op0=mybir.AluOpType.mult,
                        op1=mybir.AluOpType.add)
                return goA, goB

            def mk_v_task(mc):
                def go():
                    tag = ("projv", "projkq")[_ptag[0] % 2]
                    _ptag[0] += 1
                    ps = psum.tile([128, COLS], F32, tag=tag, bufs=1,
                                   name=f"pv{nc.next_id()}")
                    h128 = (mc % 2) * 128
                    for kc in range(KC):
                        nc.tensor.matmul(
                            ps, lhsT=x_sb[:, mc // 2, kc, h128:h128 + 128],
                            rhs=wv_sb[:, kc, :],
                            start=(kc == 0), stop=(kc == KC - 1))
                    nc.vector.tensor_tensor(
                        out=v_aug[:, mc, :, 0:64],
                        in0=ps[:, :].rearrange("p (h d) -> p h d", h=HPC),
                        in1=bv_sb.rearrange("p (h d) -> p h d", h=HPC),
                        op=mybir.AluOpType.add)
                    if apply_mask:
                        nc.vector.tensor_tensor(
                            out=v_aug[:, mc, :, :],
                            in0=v_aug[:, mc, :, :],
                            in1=mm_sb[:, mc:mc + 1, None]
                                .to_broadcast([128, HPC, 128]),
                            op=mybir.AluOpType.mult)
                return go

            def mk_v_halves(mc):
                box = {}

                def goA():
                    tag = ("projv", "projkq")[_ptag[0] % 2]
                    _ptag[0] += 1
                    ps = psum.tile([128, COLS], F32, tag=tag, bufs=1,
                                   name=f"pva{nc.next_id()}")
                    h128 = (mc % 2) * 128
                    for kc in range(4):
                        nc.tensor.matmul(
                            ps, lhsT=x_sb[:, mc // 2, kc, h128:h128 + 128],
                            rhs=wv_sb[:, kc, :],
                            start=(kc == 0), stop=(kc == 3))
                    tmp = work.tile([128, COLS], F32, tag="ptmp",
                                    name=f"vm{nc.next_id()}")
                    nc.vector.tensor_tensor(
                        out=tmp, in0=ps,
                        in1=bv_sb, op=mybir.AluOpType.add)
                    box["tmp"] = tmp

                def goB():
                    tag = ("projv", "projkq")[_ptag[0] % 2]
                    _ptag[0] += 1
                    ps = psum.tile([128, COLS], F32, tag=tag, bufs=1,
                                   name=f"pvb{nc.next_id()}")
                    h128 = (mc % 2) * 128
                    for kc in range(4, KC):
                        nc.tensor.matmul(
                            ps, lhsT=x_sb[:, mc // 2, kc, h128:h128 + 128],
                            rhs=wv_sb[:, kc, :],
                            start=(kc == 4), stop=(kc == KC - 1))
                    nc.vector.scalar_tensor_tensor(
                        out=v_aug[:, mc, :, 0:64],
                        in0=box["tmp"].rearrange("p (h d) -> p h d", h=HPC),
                        scalar=1.0,
                        in1=ps[:, :].rearrange("p (h d) -> p h d", h=HPC),
                        op0=mybir.AluOpType.mult, op1=mybir.AluOpType.add)
                    if apply_mask:
                        nc.vector.tensor_tensor(
                            out=v_aug[:, mc, :, :],
                            in0=v_aug[:, mc, :, :],
                            in1=mm_sb[:, mc:mc + 1, None]
                                .to_broadcast([128, HPC, 128]),
                            op=mybir.AluOpType.mult)
                return goA, goB

            # deferred-e store for the ramp (pair-0 qc1 exps run during qc0's
            # projection-heavy window; their ctx matmuls run later in qc2)
            e_defer = singles.tile([128, MC, 2, 512], BF16)
            e_defer0 = singles.tile([128, MC, 2, 512], BF16)

            def fill_mms(p, qc, kc, s):
                qsl = slice(qc * 512, (qc + 1) * 512)
                ksl = slice(kc * 128, (kc + 1) * 128)
                nc.tensor.matmul(s[:, 0, :], lhsT=kT[0:64, p, ksl],
                                 rhs=qT[0:64, p, qsl], start=True, stop=True)
                nc.tensor.matmul(s[:, 1, :], lhsT=kT[64:128, p, ksl],
                                 rhs=qT[64:128, p, qsl], start=True, stop=True)

            def ctx_mms(p, kc, e_ap, ctx_a, ctx_b, start, stop):
                ha, hb = 2 * p, 2 * p + 1
                nc.tensor.matmul(ctx_a, lhsT=v_aug[:, kc, ha, :],
                                 rhs=e_ap[0], start=start, stop=stop)
                nc.tensor.matmul(ctx_b, lhsT=v_aug[:, kc, hb, :],
                                 rhs=e_ap[1], start=start, stop=stop)

            def mk_ctx(p, qc):
                return (psum.tile([128, 512], F32, tag="ctx",
                                  name=f"ca{p}{qc}{nc.next_id()}"),
                        psum.tile([128, 512], F32, tag="ctx",
                                  name=f"cb{p}{qc}{nc.next_id()}"))

            import collections
            norm_q = collections.deque()

            def normalize_steps(p, qc, ctx_pair):
                """6 DVE closures (2 chains of 3) queued for spreading,
                drained two per subsequent patch, so the DVE never bursts
                and proj-slot WAR waits stay short.  Partition-shifted
                reads (den on psum rows 64:128 -> lanes 0:64) only work
                SBUF->SBUF, so the chain copies out of PSUM unshifted
                first."""
                ha = 2 * p
                qsl = slice(qc * 512, (qc + 1) * 512)
                for h, ctx in ((ha, ctx_pair[0]), (ha + 1, ctx_pair[1])):
                    box = {}

                    def s1(ctx=ctx, box=box):
                        d0 = work.tile([64, 512], F32, tag="den0",
                                       name=f"d0{nc.next_id()}")
                        nc.vector.tensor_copy(out=d0, in_=ctx[64:128, :])
                        box["d0"] = d0

                    def s2(box=box):
                        d = work.tile([64, 512], F32, tag="den",
                                      name=f"d{nc.next_id()}")
                        nc.vector.reciprocal_approx_fast(
                            out=d, in_=box["d0"])
                        box["d"] = d

                    def s3(h=h, ctx=ctx, box=box):
                        o = work.tile([64, 512], F32, tag="outt",
                                      name=f"o{nc.next_id()}")
                        nc.vector.tensor_tensor(out=o, in0=ctx[0:64, :],
                                                in1=box["d"],
                                                op=mybir.AluOpType.mult)
                        nc.sync.dma_start(out=out_ext[h][:, qsl], in_=o)

                    for s in (s1, s2, s3):
                        norm_q.append(s)

            def drain_norm(k=None):
                n = len(norm_q) if k is None else min(k, len(norm_q))
                for _ in range(n):
                    norm_q.popleft()()

            # Software-pipelined emission: PE stream per patch i is
            #   fill(i), tasks(i-1), ctx(i-2), fill(i+1), tasks(i), ctx(i-1)
            # Tasks have no exp dependency (they pre-run during exps).
            # ctx lags TWO patches so its exp semaphore is long satisfied
            # when the PE reaches it: the weight load pipelines instead of
            # serializing after the wait (~160ns/patch) and the PE never
            # idles on the exp.  e tiles are 4-buffered so a 2-patch-old e
            # is still live.
            # Deferred-emission queues: entries are (due_patch, closure).
            # tasks run one patch after their emission point (lag 1), ctx
            # pairs two (lag 2).  The queues roll ACROSS window boundaries
            # (no flush) so the PE never gets a bunched backlog that would
            # stall the next window's first fill.  A window's normalize is
            # queued INTO ctxq right behind its stop matmul: the Tile
            # framework tracks deps at emission time, so the norm reads
            # must be emitted after the accumulation's last write.
            taskq = collections.deque()
            ctxq = collections.deque()
            gp = [0]

            def flush_all():
                while taskq:
                    taskq.popleft()[1]()
                while ctxq:
                    ctxq.popleft()[1]()

            def flush_pending():
                flush_all()
                drain_norm(2)

            def patch(p, qc, kc, ctx_pair, tasks, e_dst=None):
                """ctx_pair=None -> exp only (e parked in e_dst)."""
                if e_dst is None:
                    e_dst = work.tile([128, 2, 512], BF16, tag="expT",
                                      name=f"e{nc.next_id()}")
                s = psum.tile([128, 2, 512], F32, tag="sps",
                              name=f"s{nc.next_id()}")
                fill_mms(p, qc, kc, s)
                last = kc == MC - 1 and ctx_pair is not None
                if last:
                    # final patch of a live window: emit the exp FIRST so
                    # this very patch's ctx can be emitted in-window (it
                    # may only be emitted after its exp, and it executes
                    # in the exp's shadow) -- keeps the boundary flush to
                    # just tasks(15)
                    nc.scalar.activation(e_dst, s,
                                         mybir.ActivationFunctionType.Exp,
                                         scale=0.125)
                    ctxq.append(
                        (gp[0],
                         lambda p=p, kc=kc, e_dst=e_dst, ctx_pair=ctx_pair:
                         ctx_mms(p, kc,
                                 (e_dst[:, 0, :], e_dst[:, 1, :]),
                                 *ctx_pair, start=False, stop=True)))
                while taskq and taskq[0][0] <= gp[0]:
                    taskq.popleft()[1]()
                while ctxq and ctxq[0][0] <= gp[0] + (kc == MC - 1):
                    ctxq.popleft()[1]()
                drain_norm(2)
                if not last:
                    nc.scalar.activation(e_dst, s,
                                         mybir.ActivationFunctionType.Exp,
                                         scale=0.125)
                taskq.append(
                    (gp[0] + 1, lambda tasks=tasks: [t() for t in tasks]))
                if ctx_pair is not None and not last:
                    ctxq.append(
                        (gp[0] + 2,
                         lambda p=p, kc=kc, e_dst=e_dst, ctx_pair=ctx_pair:
                         ctx_mms(p, kc,
                                 (e_dst[:, 0, :], e_dst[:, 1, :]),
                                 *ctx_pair, start=(kc == 0),
                                 stop=(kc == MC - 1))))
                gp[0] += 1

            # ---- task schedule ----
            # kq[(t, p, c256)]: 256-column kT/qT projection task
            kq = {}
            for t, dst, w_sb, b_sb in (("kT", kT, wk_sb, bk_sb),
                                       ("qT", qT, wq_sb, bq_sb)):
                for p in range(2):
                    for c in range(8):
                        kq[(t, p, c)] = mk_kqT_task(dst, w_sb, b_sb, p, c)
            v_t = {mc: mk_v_task(mc) for mc in range(MC)}

            # pair-0 minimal prefix: keys 0:256 and the qc0 queries (v is
            # first consumed in qc2 now that qc0's ctx is deferred, so v0
            # rides the ramp instead of the serial prefix)
            kq[("kT", 0, 0)]()
            kq[("qT", 0, 0)]()
            kq[("qT", 0, 1)]()

            # ---- pair-0 ramp: qc0 patches 0-4 solo, then interleave with
            # qc1 (exp only, e parked in SBUF; its ctx runs in qc2), then
            # qc1 tail.  One task per patch; each task is placed at the
            # earliest patch whose input DMA (x chunk / wv) has surely
            # landed, x-gated kT/qT chunks first (their fill deadlines are
            # hard), v chunks in the remaining slots (first consumed in
            # qc2).
            ramp = [
                (0, 0, [kq[("kT", 0, 1)]]),
                (0, 1, [kq[("kT", 0, 2)]]),
                (0, 2, [kq[("qT", 0, 2)]]),
                (0, 3, [kq[("qT", 0, 3)]]),
                (0, 4, [kq[("kT", 0, 3)]]),
            ]
            qc1_tasks = [
                [v_t[5]], [kq[("kT", 0, 4)]], [kq[("kT", 0, 5)]],
                [kq[("kT", 0, 6)]], [kq[("kT", 0, 7)]],
                [kq[("qT", 0, 4)]], [kq[("qT", 0, 5)]],
                [kq[("qT", 0, 6)]], [kq[("qT", 0, 7)]],
                [v_t[12]], [v_t[13]],
            ]
            qc0_tail = [
                [v_t[6]], [v_t[7]], [v_t[8]], [v_t[9]], [v_t[10]],
                [v_t[11]], [v_t[14]], [v_t[15]], [v_t[0]], [v_t[1]],
                [v_t[2]],
            ]
            for i in range(11):
                ramp.append((1, i, qc1_tasks[i]))
                ramp.append((0, 5 + i, qc0_tail[i]))
            for i, t in ((11, [v_t[3]]), (12, [v_t[4]]), (13, []),
                         (14, []), (15, [])):
                ramp.append((1, i, t))

            for qc, kc, tasks in ramp:
                if qc == 0:
                    patch(0, 0, kc, None, tasks,
                          e_dst=e_defer0[:, kc, :, :])
                else:
                    patch(0, 1, kc, None, tasks, e_dst=e_defer[:, kc, :, :])
            flush_pending()

            # qc2: own patches + qc1's deferred ctx matmuls (2 per patch)
            ctx1 = (psum.tile([128, 512], F32, tag="projkq", bufs=1,
                              name="dca"),
                    psum.tile([128, 512], F32, tag="projv", bufs=1,
                              name="dcb"))
            ctx2 = mk_ctx(0, 2)
            for kc in range(MC):
                patch(0, 2, kc, ctx2,
                      [lambda kc=kc: ctx_mms(
                          0, kc, (e_defer[:, kc, 0, :], e_defer[:, kc, 1, :]),
                          *ctx1, start=(kc == 0), stop=(kc == MC - 1))])
            flush_pending()
            normalize_steps(0, 2, ctx2)
            normalize_steps(0, 1, ctx1)

            # qc3: own patches + pair-1 kT + qT(1,0)
            # only the tasks pair-1 qc0 needs up-front ride here, on
            # alternating patches (consecutive task-patches overflow the
            # per-patch PE slack; alternating ones are absorbed by the
            # s-tile double buffer), late enough that the qc1/qc2
            # normalize drain (DVE) finishes first
            qc3_tasks = {5: kq[("kT", 1, 0)], 7: kq[("kT", 1, 1)],
                         9: kq[("kT", 1, 2)], 11: kq[("kT", 1, 3)],
                         13: kq[("qT", 1, 0)], 15: kq[("qT", 1, 1)]}
            ctx3 = mk_ctx(0, 3)
            for kc in range(MC):
                t = [qc3_tasks[kc]] if kc in qc3_tasks else []
                patch(0, 3, kc, ctx3, t)
            flush_pending()
            normalize_steps(0, 3, ctx3)

            # ---- pair 1 (kT tail + qT chunks ride alternating patches of
            # earlier windows; kT(1,4..7) are first needed at qc0 patch 8) ----
            p1_tasks = {
                (0, 1): [kq[("kT", 1, 4)]], (0, 3): [kq[("kT", 1, 5)]],
                (0, 5): [kq[("kT", 1, 6)]], (0, 7): [kq[("kT", 1, 7)]],
                (0, 9): [kq[("qT", 1, 2)]], (0, 11): [kq[("qT", 1, 3)]],
                (1, 1): [kq[("qT", 1, 4)]], (1, 3): [kq[("qT", 1, 5)]],
                (1, 5): [kq[("qT", 1, 6)]], (1, 7): [kq[("qT", 1, 7)]],
            }
            ctx00 = None
            for qc in range(QC):
                ctxp = mk_ctx(1, qc)
                if qc == 2:
                    # qc0's deferred ctx matmuls fill this window's PE slack
                    # (its proj psum tags are idle here)
                    ctx00 = (psum.tile([128, 512], F32, tag="projkq", bufs=1,
                                       name="d0a"),
                             psum.tile([128, 512], F32, tag="projv", bufs=1,
                                       name="d0b"))
                for kc in range(MC):
                    t = list(p1_tasks.get((qc, kc), []))
                    if qc == 2:
                        t.append(lambda kc=kc: ctx_mms(
                            0, kc,
                            (e_defer0[:, kc, 0, :], e_defer0[:, kc, 1, :]),
                            *ctx00, start=(kc == 0), stop=(kc == MC - 1)))
                    patch(1, qc, kc, ctxp, t)
                flush_pending()
                normalize_steps(1, qc, ctxp)
                if qc == 2:
                    normalize_steps(0, 0, ctx00)
            flush_all()
            drain_norm()

    nc.compile()
    return nc


def _get_nc(apply_mask: bool) -> bass.Bass:
    if apply_mask not in _CACHE:
        _CACHE[apply_mask] = build(apply_mask)
    return _CACHE[apply_mask]


def _shuf_w(w):
    # [HID, COLS] -> [2, 128, KC, 128]: pair-major, partition-contiguous
    return np.ascontiguousarray(
        w.reshape(KC, 128, 2, 128).transpose(2, 1, 0, 3)).astype(np_bf16)


def _shuf_w_flat(w):
    # [HID, COLS] -> [128, KC, COLS]: partition-contiguous DMA lines
    return np.ascontiguousarray(
        w.reshape(KC, 128, COLS).transpose(1, 0, 2)).astype(np_bf16)


def _in_maps(x, mask, Wq, bq, Wk, bk, Wv, bv, apply_mask):
    # x[b].T is [HID, S]; shuffle to [8, 128, KC, 256] (256-position
    # chunks, 1:1 with projection tasks) so each DMA reads 4KB contiguous
    # per partition and the first projections start as early as possible
    xT_b = [np.ascontiguousarray(
        x[b].T.reshape(KC, 128, 8, 256).transpose(2, 1, 0, 3)).astype(np_bf16)
        for b in range(B)]
    maps = []
    for c in range(NCORES):
        b, hg = c // 4, c % 4
        cs = slice(hg * COLS, (hg + 1) * COLS)
        m = {
            "xT": xT_b[b],
            "wq": _shuf_w(Wq[:, cs]),
            "wk": _shuf_w(Wk[:, cs]),
            "wv": _shuf_w_flat(Wv[:, cs]),
            "bq": np.ascontiguousarray(bq[cs].reshape(2, 128).T).astype(np.float32),
            "bk": np.ascontiguousarray(bk[cs].reshape(2, 128).T).astype(np.float32),
            "bv": np.ascontiguousarray(
                np.broadcast_to(bv[cs], (128, COLS))).astype(np.float32),
        }
        if apply_mask:
            m["maskm"] = np.ascontiguousarray(
                mask[b].astype(np.float32).reshape(MC, 128).T)
        maps.append(m)
    return maps


def _ensure_ntff_hook():
    """The agent image's antenv lacks axon_hooks; synthesize it so
    run_bass_kernel_spmd(trace=True) can reach the axon NTFF profiler."""
    import sys as _sys
    import types as _types
    try:
        from antenv import axon_hooks  # noqa: F401
        return
    except ImportError:
        pass
    import antenv
    mod = _types.ModuleType("antenv.axon_hooks")
    _hook = [None]
    mod.set_axon_ntff_profile_hook = lambda h: _hook.__setitem__(0, h)
    mod.get_axon_ntff_profile_hook = lambda: _hook[0]
    _sys.modules["antenv.axon_hooks"] = mod
    antenv.axon_hooks = mod
    from trn_agent_boot.trn_boot import _ntff_profile_via_ctypes
    mod.set_axon_ntff_profile_hook(
        _ntff_profile_via_ctypes("/opt/axon/libaxon_pjrt.so"))


def run(inputs: dict, trace: bool = False):
    if trace:
        _ensure_ntff_hook()
    x = np.asarray(inputs["x"], dtype=np.float32)
    mask = np.asarray(inputs["mask"])
    apply_mask = not bool((mask == 1).all())
    nc = _get_nc(apply_mask)
    maps = _in_maps(x, mask, np.asarray(inputs["Wq"], np.float32),
                    np.asarray(inputs["bq"], np.float32),
                    np.asarray(inputs["Wk"], np.float32),
                    np.asarray(inputs["bk"], np.float32),
                    np.asarray(inputs["Wv"], np.float32),
                    np.asarray(inputs["bv"], np.float32), apply_mask)
    res = run_bass_kernel_spmd(nc, maps, core_ids=list(range(NCORES)), trace=trace)
    out = np.empty((B, S, HID), dtype=np.float32)
    for c in range(NCORES):
        b, hg = c // 4, c % 4
        cs = slice(hg * COLS, (hg + 1) * COLS)
        ctxT = res.results[c]["out"]          # [HPC, D, S]
        out[b, :, cs] = ctxT.transpose(2, 0, 1).reshape(S, COLS)
    return out, res


def kernel(**inputs) -> np.ndarray:
    out, _ = run(inputs)
    return out

